# revision 23
# baseline (speedup 1.0000x reference)
"""GQA causal attention (B=2, S=2048, H=2048, 32 Q heads / 8 KV heads, hd=64)
as an 8-way batch x head tensor-parallel Trainium2 Bass kernel.

Sharding: core c = (batch b = c//4, group g = c%4). Each core gets one batch
element, 8 Q heads (two GQA groups) and their 2 KV heads; Wq/Wk/Wv column
slices, Wo row slice. Host sums the 4 partials per batch (the Wo all-reduce).

Head layout trick: per-core Q-head columns are permuted host-side so chunk cc
of qt_sb holds head cc on partitions 0-63 (KV head 0) and head 4+cc on
partitions 64-127 (KV head 1). K^T for KV head j sits on partitions 64j..64j+63
of kt_sb, so every scores matmul has matching base partitions with no K
duplication. Wo rows carry the same permutation.

Per-core dataflow (d-major; host passes hidden pre-transposed):
    Q_T  = (Wq_c * scale)^T @ hidden^T            [512, S]
    K_T  = [Wk0|Wk1]^T @ hidden^T                 [128, S]
    V_T  -> PE-transpose -> V_aug [S-chunked, 65] bf16 (ones col = denom)
    S_T[k,q] = tri_mask (identity-stationary matmul) + K_T(chunk)^T x Q_T
               (diagonal chunks first, extents trimmed to the causal region)
    P_T  = exp(S_T)  bf16                          (scores are O(+-10))
    ctx_aug = V_aug^T @ P_T                        [65, q]; row 64 = denom
    ctx  = ctx_aug[:64] * recip_approx(denom)      broadcast via gpsimd
    out_partial = ctx^T @ Wo_c                     [S, 2048], accumulated by qtb
Wo for q-block i is emitted after attention of q-block i+1 so the scheduler
keeps the PE stream dense (TRN2 PE clock ramps only while continuously busy).
"""

import sys

for _p in ("/root/.axon_site", "/root/.axon_site/_ro/trn_rl_repo",
           "/root/.axon_site/_ro/pypackages", "/opt/trn_rl_repo", "/opt/pypackages"):
    if _p not in sys.path:
        sys.path.append(_p)

from contextlib import ExitStack

import numpy as np

import concourse.bass as bass  # noqa: F401
import concourse.tile as tile
from concourse import bacc, mybir
from concourse.bass_utils import run_bass_kernel_spmd

F32 = mybir.dt.float32
F32R = mybir.dt.float32r
BF16 = mybir.dt.bfloat16
P = 128
KC = 128
QT = 512
N_CORES = 8
HD = 64
NEG = -1e9

TRACE = False            # test harness flips this for NTFF profiling
TRACE_CORES = None
LAST_RESULT = None       # BassKernelResults of the last run (for the harness)

_nc_cache = {}


def build_attn_core(S=2048, H=2048, NH=8, mask_mode="causal", debug_dump=False):
    """Build + bass-compile the per-core program (one batch element).

    DRAM inputs (per core):
      ht  [H, S] f32r    hidden transposed       wq [H, NH*HD] f32r (pre-scaled,
                                                 head-permuted: chunk cc = heads
                                                 (cc, 4+cc))
      wkv [H, 4*HD] f32r [K0|K1|V0|V1]           wo [NH*HD, H] f32r (row-permuted)
      trif [KC, QT] f32r [tri | zeros]: trif[k,j] = -1e9 if k > j else 0
      identr [P, P] f32r identity                zrow [1, HD+1] f32r zeros
      maskt [S, S] f32   (mask_mode=="full" only) additive mask transposed
    Output: out_p [S, H] f32.
    """
    NKV = 2
    CL = NH * HD                       # 512 q cols
    NCC = CL // P                      # 4 qt chunks
    NHC = H // P                       # 16 contraction chunks
    NT = S // QT                       # 4 token tiles / q blocks
    KPB = S // KC                      # 16 k chunks
    DPT = QT // KC                     # 4 k chunks per q block
    assert NH == 8 and S % QT == 0 and H % P == 0

    nc = bacc.Bacc("TRN2", target_bir_lowering=False, debug=False)

    ht = nc.dram_tensor("ht", [H, S], F32R, kind="ExternalInput").ap()
    wq = nc.dram_tensor("wq", [H, CL], F32R, kind="ExternalInput").ap()
    wkv = nc.dram_tensor("wkv", [H, 4 * HD], F32R, kind="ExternalInput").ap()
    wo = nc.dram_tensor("wo", [CL, H], F32R, kind="ExternalInput").ap()
    trif = nc.dram_tensor("trif", [KC, QT], F32R, kind="ExternalInput").ap()
    identr = nc.dram_tensor("identr", [P, P], F32R, kind="ExternalInput").ap()
    zrow = nc.dram_tensor("zrow", [1, HD + 1], F32R, kind="ExternalInput").ap()
    if mask_mode == "full":
        maskt = nc.dram_tensor("maskt", [S, S], F32, kind="ExternalInput").ap()
    out_p = nc.dram_tensor("out_p", [S, H], F32, kind="ExternalOutput").ap()
    if debug_dump:
        dbg_qt = nc.dram_tensor("dbg_qt", [NCC, P, S], F32, kind="ExternalOutput").ap()
        dbg_kt = nc.dram_tensor("dbg_kt", [P, S], F32, kind="ExternalOutput").ap()
        dbg_v = nc.dram_tensor("dbg_v", [P, S // KC, NKV, HD + 1], F32,
                               kind="ExternalOutput").ap()
        dbg_ctx = nc.dram_tensor("dbg_ctx", [2, P, NCC, QT], F32,
                                 kind="ExternalOutput").ap()

    with tile.TileContext(nc) as tc, ExitStack() as ctx:
        # ---- persistent SBUF ----
        pers = ctx.enter_context(tc.tile_pool(name="pers", bufs=1))
        wq_sb = pers.tile([P, NHC, CL], F32R, tag="wq")
        wkv_sb = pers.tile([P, NHC, 4 * HD], F32R, tag="wkv")
        # DMA order matters: the first proj matmul needs only wq/wkv chunk 0
        # (+ its h_t tile) — emit those first so the PE starts ~2us in, and
        # the big wo_sb transfer last (not needed until attention ends).
        wq_r = wq.rearrange("(o p) m -> p o m", p=P)
        wkv_r = wkv.rearrange("(o p) m -> p o m", p=P)
        trif_sb = pers.tile([KC, QT], F32R, tag="trif")
        idr_sb = pers.tile([P, P], F32R, tag="identr")
        zrow_sb = pers.tile([1, HD + 1], F32R, tag="zrow")
        wo_sb = pers.tile([P, NCC, H], F32R, tag="wo")

        def late_weight_dmas():
            nc.sync.dma_start(trif_sb[:], trif)
            nc.sync.dma_start(idr_sb[:], identr)
            nc.sync.dma_start(zrow_sb[:], zrow)
            for oc in range(NCC):
                nc.sync.dma_start(wo_sb[:, oc], wo.rearrange(
                    "(o p) m -> p o m", p=P)[:, oc])

        # fp32 identity for PE transposes (gpsimd memset can write f32)
        ident = pers.tile([P, P], F32, tag="ident")
        nc.gpsimd.memset(ident[:], 1.0)
        nc.gpsimd.affine_select(
            out=ident[:], in_=ident[:],
            compare_op=mybir.AluOpType.is_equal, fill=0.0,
            base=0, pattern=[[-1, P]], channel_multiplier=1,
        )

        qt_sb = pers.tile([P, NCC, S], F32R, tag="qt")
        kt_sb = pers.tile([P, S], F32R, tag="kt")
        v_sb = pers.tile([P, KPB, NKV, HD + 1], BF16, tag="v")
        # ones column for the softmax denominator: fill everything with 1.0,
        # proj drains then overwrite cols 0..63
        nc.gpsimd.memset(v_sb[:], 1.0)

        # ---- pools ----
        hpool = ctx.enter_context(tc.tile_pool(name="hpool", bufs=4))
        vtmp_pool = ctx.enter_context(tc.tile_pool(name="vtmp", bufs=2))
        pt_pool = ctx.enter_context(tc.tile_pool(name="pt", bufs=4))
        npool = ctx.enter_context(tc.tile_pool(name="npool", bufs=3))
        cxpool = ctx.enter_context(tc.tile_pool(name="cxpool", bufs=2))
        if mask_mode == "full":
            mpool = ctx.enter_context(tc.tile_pool(name="mpool", bufs=4))

        # PSUM: exactly 8 banks across both phases
        psS = ctx.enter_context(tc.tile_pool(name="psS", bufs=2, space="PSUM"))
        psC = ctx.enter_context(tc.tile_pool(name="psC", bufs=2, space="PSUM"))
        psO = ctx.enter_context(tc.tile_pool(name="psO", bufs=2, space="PSUM"))

        ctx_sb = [cxpool.tile([P, NCC, QT], F32R, tag="ctx", name=f"ctx{i}")
                  for i in range(2)]

        # ================= projections (one q tile) =================
        # psS slots ([128,1024] = 2 banks) hold Q chunk pairs; psO slots
        # ([128,512]) hold the K and V chunks; psC slots host the transposes.
        # Tiles >= 1 are emitted interleaved with attention of tile-1 (their
        # drains go on DVE; Act is the attention-window co-bottleneck).
        def proj_tile(t):
            q0 = t * QT
            pq01 = psS.tile([P, 2 * QT], F32, tag="sps", name="pq01")
            pq23 = psS.tile([P, 2 * QT], F32, tag="sps", name="pq23")
            pk = psO.tile([P, QT], F32, tag="po", name="pk")
            pv = psO.tile([P, QT], F32, tag="po", name="pv")
            for hc in range(NHC):
                if t == 0:
                    # stream the weight chunks in with the first tile so the
                    # PE starts ~2us in instead of waiting for all weights
                    nc.sync.dma_start(wq_sb[:, hc], wq_r[:, hc])
                    nc.sync.dma_start(wkv_sb[:, hc], wkv_r[:, hc])
                h_t = hpool.tile([P, QT], F32R, tag="h")
                nc.sync.dma_start(h_t[:], ht[hc * P:(hc + 1) * P, q0:q0 + QT])
                fl = dict(start=(hc == 0), stop=(hc == NHC - 1))
                nc.tensor.matmul(pq01[:, :QT], wq_sb[:, hc, 0:P], h_t[:], **fl)
                nc.tensor.matmul(pq01[:, QT:], wq_sb[:, hc, P:2 * P], h_t[:], **fl)
                nc.tensor.matmul(pq23[:, :QT], wq_sb[:, hc, 2 * P:3 * P], h_t[:], **fl)
                nc.tensor.matmul(pq23[:, QT:], wq_sb[:, hc, 3 * P:4 * P], h_t[:], **fl)
                nc.tensor.matmul(pk[:], wkv_sb[:, hc, 0:P], h_t[:], **fl)
                nc.tensor.matmul(pv[:], wkv_sb[:, hc, P:2 * P], h_t[:], **fl)
            cp = nc.scalar.copy if t == 0 else nc.vector.tensor_copy
            cp(qt_sb[:, 0, q0:q0 + QT], pq01[:, :QT])
            cp(qt_sb[:, 1, q0:q0 + QT], pq01[:, QT:])
            cp(qt_sb[:, 2, q0:q0 + QT], pq23[:, :QT])
            cp(qt_sb[:, 3, q0:q0 + QT], pq23[:, QT:])
            cp(kt_sb[:, q0:q0 + QT], pk[:])
            vtmp = vtmp_pool.tile([P, QT], F32, tag="vt")
            nc.vector.tensor_copy(vtmp[:], pv[:])
            # V_T chunk -> PE-transpose into v_sb (natural [k, d] layout)
            for j in range(NKV):
                for s4 in range(DPT):
                    tp = psC.tile([P, HD], F32, tag="cps", name="tp")
                    nc.tensor.transpose(
                        tp[:, :HD],
                        vtmp[HD * j:HD * (j + 1), s4 * KC:(s4 + 1) * KC],
                        ident[HD * j:HD * (j + 1), HD * j:HD * (j + 1)],
                    )
                    nc.vector.tensor_copy(v_sb[:, t * DPT + s4, j, :HD], tp[:, :HD])

        def debug_dumps():
            for c in range(NCC):
                nc.sync.dma_start(dbg_qt[c], qt_sb[:, c, :].bitcast(F32))
            nc.sync.dma_start(dbg_kt[:], kt_sb[:].bitcast(F32))
            dpool = ctx.enter_context(tc.tile_pool(name="dpool", bufs=1))
            dvt = dpool.tile([P, KPB, NKV, HD + 1], F32, tag="dvt")
            nc.vector.tensor_copy(dvt[:], v_sb[:])
            nc.sync.dma_start(dbg_v[:], dvt[:])

        # ================= Phase B: attention, Wo lagged one q-block =======
        def emit_wo(qtb):
            src = ctx_sb[qtb % 2]
            q0 = qtb * QT
            for tc_ in range(QT // P):
                for et in range(H // QT):
                    po = psO.tile([P, QT], F32, tag="po", name="po")
                    for cc in range(NCC):
                        nc.tensor.matmul(
                            po[:],
                            src[:, cc, tc_ * P:(tc_ + 1) * P],
                            wo_sb[:, cc, et * QT:(et + 1) * QT],
                            start=(cc == 0), stop=(cc == NCC - 1),
                        )
                    ob = npool.tile([P, QT], F32, tag="ob")
                    nc.vector.tensor_copy(ob[:], po[:])
                    nc.sync.dma_start(
                        out_p[q0 + tc_ * P:q0 + (tc_ + 1) * P,
                              et * QT:(et + 1) * QT],
                        ob[:],
                    )

        def chunk_pairs(qtb):
            if mask_mode == "causal":
                # (kc, local q offset, extent, in-tile offset); diagonal
                # chunks first, paired (d0,d3) and (d1,d2) so each pair's
                # exp covers ONE contiguous range
                d = qtb * DPT
                pairs = [
                    [(d + 0, 0, QT, 0), (d + 3, 3 * KC, KC, QT)],
                    [(d + 1, KC, QT - KC, KC), (d + 2, 2 * KC, 2 * KC, QT)],
                ]
                ints = [(kc, 0, QT, (i % 2) * QT)
                        for i, kc in enumerate(range(qtb * DPT))]
            else:
                ints = [(kc, 0, QT, (i % 2) * QT)
                        for i, kc in enumerate(range(KPB))]
                pairs = []
            return pairs + [ints[i:i + 2] for i in range(0, len(ints), 2)]

        def emit_pair(qtb, h, cps, pair, first, lastp):
            q0 = qtb * QT
            r, cc = h // 4, h % 4
            rows = slice(HD * r, HD * (r + 1))
            sps = psS.tile([P, 2 * QT], F32, tag="sps", name="sps")
            pt = pt_pool.tile([P, 2 * QT], BF16, tag="pt")
            for kc, qoff, ext, off in pair:
                diag = mask_mode == "causal" and kc >= qtb * DPT
                if diag:
                    # tri mask first (start=True zeroes the region),
                    # then accumulate the scores on top
                    nc.tensor.matmul(
                        sps[:, off:off + ext], idr_sb[:],
                        trif_sb[:, :ext], start=True, stop=False)
                nc.tensor.matmul(
                    sps[:, off:off + ext],
                    kt_sb[rows, kc * KC:(kc + 1) * KC],
                    qt_sb[rows, cc, q0 + qoff:q0 + qoff + ext],
                    start=not diag, stop=True,
                )
                if mask_mode == "full":
                    mt = mpool.tile([KC, QT], F32, tag="mt")
                    nc.sync.dma_start(
                        mt[:], maskt[kc * KC:(kc + 1) * KC, q0:q0 + QT])
                    nc.vector.tensor_add(sps[:, off:off + ext],
                                         sps[:, off:off + ext], mt[:])
            # one exp over the pair's contiguous range
            lo = min(off for _, _, _, off in pair)
            hi = max(off + ext for _, _, ext, off in pair)
            nc.scalar.activation(pt[:, lo:hi], sps[:, lo:hi],
                                 mybir.ActivationFunctionType.Exp)
            for ci, (kc, qoff, ext, off) in enumerate(pair):
                nc.tensor.matmul(
                    cps[:, qoff:qoff + ext],
                    v_sb[:, kc, r, :],
                    pt[:, off:off + ext],
                    start=first and ci == 0,
                    stop=lastp and ci == len(pair) - 1 and qoff == 0,
                )

        def finish_head(qtb, h, cps):
            q0 = qtb * QT
            r, cc = h // 4, h % 4
            if mask_mode == "causal" and qtb == 0:
                # no full-width final chunk at qtb 0; close the accumulation
                # group over the whole bank with a zero matmul
                nc.tensor.matmul(
                    cps[:], zrow_sb[:], qt_sb[0:1, 0, q0:q0 + QT],
                    start=False, stop=True)
            # ---- normalize: ctx[:64] * recip(denom row 64) ----
            # single Act copy moves ctx+denom off PSUM at once: cps releases
            # ~1us after the last AV matmul instead of after the whole
            # normalize chain (whose DMA completion sems cost ~900ns each),
            # which kept head-of-line-blocking the next head's AV matmuls
            ct = npool.tile([HD + 1, QT], F32, tag="ctf")
            nc.vector.tensor_copy(ct[:], cps[:])
            recip = npool.tile([P, QT + 2 * (QT // P)], F32, tag="recip")
            DW = QT // P
            # reshape the [1, QT] denom row to [P, QT/P] via DMA so the
            # partition-serial reciprocal runs 128-wide (~100ns vs ~4us)
            nc.sync.dma_start(recip[:, QT:QT + DW], ct[HD:HD + 1, :])
            nc.vector.reciprocal(recip[:, QT + DW:], recip[:, QT:QT + DW])
            nc.sync.dma_start(recip[0:1, :QT], recip[:, QT + DW:])
            bcast = npool.tile([HD, QT], F32, tag="bcast")
            nc.gpsimd.partition_broadcast(bcast[:], recip[0:1, :QT])
            dst = ctx_sb[qtb % 2]
            if r == 0:
                nc.vector.tensor_mul(dst[:HD, cc, :], ct[:HD, :], bcast[:])
            else:
                ctmp = npool.tile([HD, QT], F32R, tag="ctmp")
                nc.vector.tensor_mul(ctmp[:], ct[:HD, :], bcast[:])
                nc.sync.dma_start(dst[HD:2 * HD, cc, :], ctmp[:])

        def attn_head_pair(qtb, ha, hb):
            # interleave two heads' chunk streams so independent work sits
            # between each exp and the matmuls that consume it (keeps the
            # in-order PE queue from head-of-line blocking on Act latency)
            pairs = chunk_pairs(qtb)
            cpsA = psC.tile([HD + 1, QT], F32, tag="cps", name="cpsA")
            cpsB = psC.tile([HD + 1, QT], F32, tag="cps", name="cpsB")
            for p, pair in enumerate(pairs):
                emit_pair(qtb, ha, cpsA, pair, p == 0, p == len(pairs) - 1)
                emit_pair(qtb, hb, cpsB, pair, p == 0, p == len(pairs) - 1)
            finish_head(qtb, ha, cpsA)
            finish_head(qtb, hb, cpsB)

        proj_tile(0)
        late_weight_dmas()
        for qtb in range(NT):
            for hp in range(0, NH, 2):
                attn_head_pair(qtb, hp, hp + 1)
            if debug_dump and qtb < 2:
                nc.sync.dma_start(dbg_ctx[qtb], ctx_sb[qtb % 2][:].bitcast(F32))
            if qtb > 0:
                emit_wo(qtb - 1)
            if qtb + 1 < NT:
                proj_tile(qtb + 1)
        emit_wo(NT - 1)
        if debug_dump:
            debug_dumps()

    nc.compile()
    return nc


def _detect_mask_mode(m, S):
    if not np.any(m):
        return "zeros"
    b0 = np.asarray(m[0, 0])
    qi = np.arange(S)
    tl = qi[None, :] <= qi[:, None]
    if (b0[tl] == 0.0).all() and (b0[~tl] <= -1e8).all() and (m == b0).all():
        return "causal"
    return "full"


def shard_inputs(hidden_states, attention_mask, Wq, Wk, Wv, Wo, mask_mode):
    B, S, H = hidden_states.shape
    NH = Wq.shape[1] // HD             # 32 total
    NKV = Wk.shape[1] // HD            # 8 total
    G = 4                              # head-groups (cores per batch)
    NHL = NH // G                      # 8 q heads per core
    scale = np.float32(1.0 / np.sqrt(HD))

    # permutation: chunk cc holds q-heads (cc, 4+cc) -> [0,4,1,5,2,6,3,7]
    perm = [h for cc in range(NHL // 2) for h in (cc, cc + NHL // 2)]

    trif = np.zeros((KC, QT), np.float32)
    ki, qj = np.meshgrid(np.arange(KC), np.arange(KC), indexing="ij")
    trif[:, :KC] = np.where(ki > qj, NEG, 0.0).astype(np.float32)
    identr = np.eye(P, dtype=np.float32)
    zr = np.zeros((1, HD + 1), np.float32)

    in_maps = []
    for c in range(N_CORES):
        b, g = divmod(c, G)
        ht = np.ascontiguousarray(
            np.asarray(hidden_states[b]).T.astype(np.float32))
        heads = [g * NHL + perm[i] for i in range(NHL)]
        wq_c = np.ascontiguousarray(np.concatenate(
            [Wq[:, h * HD:(h + 1) * HD] for h in heads], axis=1)
            .astype(np.float32) * scale)
        kv0 = g * 2
        wkv_c = np.ascontiguousarray(np.concatenate(
            [Wk[:, kv0 * HD:(kv0 + 2) * HD],
             Wv[:, kv0 * HD:(kv0 + 2) * HD]], axis=1).astype(np.float32))
        wo_c = np.ascontiguousarray(np.concatenate(
            [Wo[h * HD:(h + 1) * HD, :] for h in heads], axis=0)
            .astype(np.float32))
        im = {"ht": ht, "wq": wq_c, "wkv": wkv_c, "wo": wo_c,
              "trif": trif, "identr": identr, "zrow": zr}
        if mask_mode == "full":
            im["maskt"] = np.ascontiguousarray(
                np.asarray(attention_mask)[b, 0].T.astype(np.float32))
        in_maps.append(im)
    return in_maps


def kernel(hidden_states, attention_mask, Wq, Wk, Wv, Wo):
    global LAST_RESULT
    hidden_states = np.asarray(hidden_states, dtype=np.float32)
    attention_mask = np.asarray(attention_mask, dtype=np.float32)
    Wq, Wk, Wv, Wo = (np.asarray(w, dtype=np.float32) for w in (Wq, Wk, Wv, Wo))
    B, S, H = hidden_states.shape

    mask_mode = _detect_mask_mode(attention_mask, S)
    in_maps = shard_inputs(hidden_states, attention_mask, Wq, Wk, Wv, Wo,
                           mask_mode)

    key = (B, S, H, mask_mode)
    if key not in _nc_cache:
        _nc_cache[key] = build_attn_core(S=S, H=H, NH=8, mask_mode=mask_mode)
    nc = _nc_cache[key]

    res = run_bass_kernel_spmd(nc, in_maps, core_ids=list(range(N_CORES)),
                               trace=TRACE, trace_cores=TRACE_CORES)
    LAST_RESULT = res

    out = np.zeros((B, S, H), np.float32)
    for c in range(N_CORES):
        out[c // 4] += res.results[c]["out_p"]
    return out


# revision 28
# speedup vs baseline: 1.1427x; 1.1427x over previous
"""GQA causal attention (B=2, S=2048, H=2048, 32 Q heads / 8 KV heads, hd=64)
as an 8-way batch x head tensor-parallel Trainium2 Bass kernel.

Sharding: core c = (batch b = c//4, group g = c%4). Each core gets one batch
element, 8 Q heads (two GQA groups) and their 2 KV heads; Wq/Wk/Wv column
slices, Wo row slice. Host sums the 4 partials per batch (the Wo all-reduce).

Head layout trick: per-core Q-head columns are permuted host-side so chunk cc
of qt_sb holds head cc on partitions 0-63 (KV head 0) and head 4+cc on
partitions 64-127 (KV head 1). K^T for KV head j sits on partitions 64j..64j+63
of kt_sb, so every scores matmul has matching base partitions with no K
duplication. Wo rows carry the same permutation.

Per-core dataflow (d-major; host passes hidden pre-transposed):
    Q_T  = (Wq_c * scale)^T @ hidden^T            [512, S]
    K_T  = [Wk0|Wk1]^T @ hidden^T                 [128, S]
    V_T  -> PE-transpose -> V_aug [S-chunked, 65] bf16 (ones col = denom)
    S_T[k,q] = tri_mask (identity-stationary matmul) + K_T(chunk)^T x Q_T
               (diagonal chunks first, extents trimmed to the causal region)
    P_T  = exp(S_T)  bf16                          (scores are O(+-10))
    ctx_aug = V_aug^T @ P_T                        [65, q]; row 64 = denom
    ctx  = ctx_aug[:64] * recip_approx(denom)      broadcast via gpsimd
    out_partial = ctx^T @ Wo_c                     [S, 2048], accumulated by qtb
Wo for q-block i is emitted after attention of q-block i+1 so the scheduler
keeps the PE stream dense (TRN2 PE clock ramps only while continuously busy).
"""

import sys

for _p in ("/root/.axon_site", "/root/.axon_site/_ro/trn_rl_repo",
           "/root/.axon_site/_ro/pypackages", "/opt/trn_rl_repo", "/opt/pypackages"):
    if _p not in sys.path:
        sys.path.append(_p)

from contextlib import ExitStack

import numpy as np

import concourse.bass as bass  # noqa: F401
import concourse.tile as tile
from concourse import bacc, mybir
from concourse.bass_utils import run_bass_kernel_spmd

F32 = mybir.dt.float32
F32R = mybir.dt.float32r
BF16 = mybir.dt.bfloat16
P = 128
KC = 128
QT = 512
N_CORES = 8
HD = 64
NEG = -1e9

TRACE = False            # test harness flips this for NTFF profiling
TRACE_CORES = None
LAST_RESULT = None       # BassKernelResults of the last run (for the harness)

_nc_cache = {}


def build_attn_core(S=2048, H=2048, NH=8, mask_mode="causal", debug_dump=False):
    """Build + bass-compile the per-core program (one batch element).

    DRAM inputs (per core):
      ht  [H, S] f32r    hidden transposed       wq [H, NH*HD] f32r (pre-scaled,
                                                 head-permuted: chunk cc = heads
                                                 (cc, 4+cc))
      wkv [H, 4*HD] f32r [K0|K1|V0|V1]           wo [NH*HD, H] f32r (row-permuted)
      trif [KC, QT] f32r [tri | zeros]: trif[k,j] = -1e9 if k > j else 0
      identr [P, P] f32r identity                zrow [1, HD+1] f32r zeros
      maskt [S, S] f32   (mask_mode=="full" only) additive mask transposed
    Output: out_p [S, H] f32.
    """
    NKV = 2
    CL = NH * HD                       # 512 q cols
    NCC = CL // P                      # 4 qt chunks
    NHC = H // P                       # 16 contraction chunks
    NT = S // QT                       # 4 token tiles / q blocks
    KPB = S // KC                      # 16 k chunks
    DPT = QT // KC                     # 4 k chunks per q block
    assert NH == 8 and S % QT == 0 and H % P == 0

    nc = bacc.Bacc("TRN2", target_bir_lowering=False, debug=False)

    ht = nc.dram_tensor("ht", [H, S], F32R, kind="ExternalInput").ap()
    wq = nc.dram_tensor("wq", [H, CL], F32R, kind="ExternalInput").ap()
    wkv = nc.dram_tensor("wkv", [H, 4 * HD], F32R, kind="ExternalInput").ap()
    wo = nc.dram_tensor("wo", [CL, H], F32R, kind="ExternalInput").ap()
    trif = nc.dram_tensor("trif", [KC, QT], F32R, kind="ExternalInput").ap()
    identr = nc.dram_tensor("identr", [P, P], F32R, kind="ExternalInput").ap()
    zrow = nc.dram_tensor("zrow", [1, HD + 1], F32R, kind="ExternalInput").ap()
    if mask_mode == "full":
        maskt = nc.dram_tensor("maskt", [S, S], F32, kind="ExternalInput").ap()
    out_p = nc.dram_tensor("out_p", [S, H], F32, kind="ExternalOutput").ap()
    if debug_dump:
        dbg_qt = nc.dram_tensor("dbg_qt", [NCC, P, S], F32, kind="ExternalOutput").ap()
        dbg_kt = nc.dram_tensor("dbg_kt", [P, S], F32, kind="ExternalOutput").ap()
        dbg_v = nc.dram_tensor("dbg_v", [P, S // KC, NKV, HD + 1], F32,
                               kind="ExternalOutput").ap()
        dbg_ctx = nc.dram_tensor("dbg_ctx", [2, P, NCC, QT], F32,
                                 kind="ExternalOutput").ap()

    with tile.TileContext(nc) as tc, ExitStack() as ctx:
        # ---- persistent SBUF ----
        pers = ctx.enter_context(tc.tile_pool(name="pers", bufs=1))
        wq_sb = pers.tile([P, NHC, CL], F32R, tag="wq")
        wkv_sb = pers.tile([P, NHC, 4 * HD], F32R, tag="wkv")
        # DMA order matters: the first proj matmul needs only wq/wkv chunk 0
        # (+ its h_t tile) — emit those first so the PE starts ~2us in, and
        # the big wo_sb transfer last (not needed until attention ends).
        wq_r = wq.rearrange("(o p) m -> p o m", p=P)
        wkv_r = wkv.rearrange("(o p) m -> p o m", p=P)
        trif_sb = pers.tile([KC, QT], F32R, tag="trif")
        idr_sb = pers.tile([P, P], F32R, tag="identr")
        zrow_sb = pers.tile([1, HD + 1], F32R, tag="zrow")
        wo_sb = pers.tile([P, NCC, H], F32R, tag="wo")

        def late_weight_dmas():
            nc.sync.dma_start(trif_sb[:], trif)
            nc.sync.dma_start(idr_sb[:], identr)
            nc.sync.dma_start(zrow_sb[:], zrow)
            for oc in range(NCC):
                nc.sync.dma_start(wo_sb[:, oc], wo.rearrange(
                    "(o p) m -> p o m", p=P)[:, oc])

        # fp32 identity for PE transposes (gpsimd memset can write f32)
        ident = pers.tile([P, P], F32, tag="ident")
        nc.gpsimd.memset(ident[:], 1.0)
        nc.gpsimd.affine_select(
            out=ident[:], in_=ident[:],
            compare_op=mybir.AluOpType.is_equal, fill=0.0,
            base=0, pattern=[[-1, P]], channel_multiplier=1,
        )

        qt_sb = pers.tile([P, NCC, S], F32R, tag="qt")
        kt_sb = pers.tile([P, S], F32R, tag="kt")
        v_sb = pers.tile([P, KPB, NKV, HD + 1], BF16, tag="v")
        # ones column for the softmax denominator: fill everything with 1.0,
        # proj drains then overwrite cols 0..63
        nc.gpsimd.memset(v_sb[:], 1.0)

        # ---- pools ----
        hpool = ctx.enter_context(tc.tile_pool(name="hpool", bufs=4))
        vtmp_pool = ctx.enter_context(tc.tile_pool(name="vtmp", bufs=2))
        pt_pool = ctx.enter_context(tc.tile_pool(name="pt", bufs=6))
        npool = ctx.enter_context(tc.tile_pool(name="npool", bufs=3))
        cxpool = ctx.enter_context(tc.tile_pool(name="cxpool", bufs=2))
        if mask_mode == "full":
            mpool = ctx.enter_context(tc.tile_pool(name="mpool", bufs=4))

        # PSUM: exactly 8 banks across both phases
        psS = ctx.enter_context(tc.tile_pool(name="psS", bufs=2, space="PSUM"))
        psC = ctx.enter_context(tc.tile_pool(name="psC", bufs=2, space="PSUM"))
        psO = ctx.enter_context(tc.tile_pool(name="psO", bufs=2, space="PSUM"))

        ctx_sb = [cxpool.tile([P, NCC, QT], F32R, tag="ctx", name=f"ctx{i}")
                  for i in range(2)]

        # ================= projections (one q tile) =================
        # psS slots ([128,1024] = 2 banks) hold Q chunk pairs; psO slots
        # ([128,512]) hold the K and V chunks; psC slots host the transposes.
        # Tiles >= 1 are emitted interleaved with attention of tile-1 (their
        # drains go on DVE; Act is the attention-window co-bottleneck).
        def proj_tile(t):
            q0 = t * QT
            pq01 = psS.tile([P, 2 * QT], F32, tag="sps", name="pq01")
            pq23 = psS.tile([P, 2 * QT], F32, tag="sps", name="pq23")
            pk = psO.tile([P, QT], F32, tag="po", name="pk")
            pv = psO.tile([P, QT], F32, tag="po", name="pv")
            for hc in range(NHC):
                if t == 0:
                    # stream the weight chunks in with the first tile so the
                    # PE starts ~2us in instead of waiting for all weights
                    nc.sync.dma_start(wq_sb[:, hc], wq_r[:, hc])
                    nc.sync.dma_start(wkv_sb[:, hc], wkv_r[:, hc])
                h_t = hpool.tile([P, QT], F32R, tag="h")
                nc.sync.dma_start(h_t[:], ht[hc * P:(hc + 1) * P, q0:q0 + QT])
                fl = dict(start=(hc == 0), stop=(hc == NHC - 1))
                nc.tensor.matmul(pq01[:, :QT], wq_sb[:, hc, 0:P], h_t[:], **fl)
                nc.tensor.matmul(pq01[:, QT:], wq_sb[:, hc, P:2 * P], h_t[:], **fl)
                nc.tensor.matmul(pq23[:, :QT], wq_sb[:, hc, 2 * P:3 * P], h_t[:], **fl)
                nc.tensor.matmul(pq23[:, QT:], wq_sb[:, hc, 3 * P:4 * P], h_t[:], **fl)
                nc.tensor.matmul(pk[:], wkv_sb[:, hc, 0:P], h_t[:], **fl)
                nc.tensor.matmul(pv[:], wkv_sb[:, hc, P:2 * P], h_t[:], **fl)
            cp = nc.scalar.copy if t == 0 else nc.vector.tensor_copy
            cp(qt_sb[:, 0, q0:q0 + QT], pq01[:, :QT])
            cp(qt_sb[:, 1, q0:q0 + QT], pq01[:, QT:])
            cp(qt_sb[:, 2, q0:q0 + QT], pq23[:, :QT])
            cp(qt_sb[:, 3, q0:q0 + QT], pq23[:, QT:])
            cp(kt_sb[:, q0:q0 + QT], pk[:])
            vtmp = vtmp_pool.tile([P, QT], F32, tag="vt")
            nc.vector.tensor_copy(vtmp[:], pv[:])
            # V_T chunk -> PE-transpose into v_sb (natural [k, d] layout)
            for j in range(NKV):
                for s4 in range(DPT):
                    tp = psC.tile([P, HD], F32, tag="cps", name="tp")
                    nc.tensor.transpose(
                        tp[:, :HD],
                        vtmp[HD * j:HD * (j + 1), s4 * KC:(s4 + 1) * KC],
                        ident[HD * j:HD * (j + 1), HD * j:HD * (j + 1)],
                    )
                    nc.vector.tensor_copy(v_sb[:, t * DPT + s4, j, :HD], tp[:, :HD])

        def debug_dumps():
            for c in range(NCC):
                nc.sync.dma_start(dbg_qt[c], qt_sb[:, c, :].bitcast(F32))
            nc.sync.dma_start(dbg_kt[:], kt_sb[:].bitcast(F32))
            dpool = ctx.enter_context(tc.tile_pool(name="dpool", bufs=1))
            dvt = dpool.tile([P, KPB, NKV, HD + 1], F32, tag="dvt")
            nc.vector.tensor_copy(dvt[:], v_sb[:])
            nc.sync.dma_start(dbg_v[:], dvt[:])

        # ================= Phase B: attention, Wo lagged one q-block =======
        def emit_wo(qtb):
            src = ctx_sb[qtb % 2]
            q0 = qtb * QT
            for tc_ in range(QT // P):
                for et in range(H // QT):
                    po = psO.tile([P, QT], F32, tag="po", name="po")
                    for cc in range(NCC):
                        nc.tensor.matmul(
                            po[:],
                            src[:, cc, tc_ * P:(tc_ + 1) * P],
                            wo_sb[:, cc, et * QT:(et + 1) * QT],
                            start=(cc == 0), stop=(cc == NCC - 1),
                        )
                    ob = npool.tile([P, QT], F32, tag="ob")
                    nc.vector.tensor_copy(ob[:], po[:])
                    nc.sync.dma_start(
                        out_p[q0 + tc_ * P:q0 + (tc_ + 1) * P,
                              et * QT:(et + 1) * QT],
                        ob[:],
                    )

        def chunk_pairs(qtb):
            if mask_mode == "causal":
                # (kc, local q offset, extent, in-tile offset); diagonal
                # chunks first, paired (d0,d3) and (d1,d2) so each pair's
                # exp covers ONE contiguous range
                d = qtb * DPT
                pairs = [
                    [(d + 0, 0, QT, 0), (d + 3, 3 * KC, KC, QT)],
                    [(d + 1, KC, QT - KC, KC), (d + 2, 2 * KC, 2 * KC, QT)],
                ]
                ints = [(kc, 0, QT, (i % 2) * QT)
                        for i, kc in enumerate(range(qtb * DPT))]
            else:
                ints = [(kc, 0, QT, (i % 2) * QT)
                        for i, kc in enumerate(range(KPB))]
                pairs = []
            return pairs + [ints[i:i + 2] for i in range(0, len(ints), 2)]

        def emit_scores_exp(qtb, h, pair):
            q0 = qtb * QT
            r, cc = h // 4, h % 4
            rows = slice(HD * r, HD * (r + 1))
            sps = psS.tile([P, 2 * QT], F32, tag="sps", name="sps")
            pt = pt_pool.tile([P, 2 * QT], BF16, tag="pt")
            for kc, qoff, ext, off in pair:
                diag = mask_mode == "causal" and kc >= qtb * DPT
                if diag:
                    # tri mask first (start=True zeroes the region),
                    # then accumulate the scores on top
                    nc.tensor.matmul(
                        sps[:, off:off + ext], idr_sb[:],
                        trif_sb[:, :ext], start=True, stop=False)
                nc.tensor.matmul(
                    sps[:, off:off + ext],
                    kt_sb[rows, kc * KC:(kc + 1) * KC],
                    qt_sb[rows, cc, q0 + qoff:q0 + qoff + ext],
                    start=not diag, stop=True,
                )
                if mask_mode == "full":
                    mt = mpool.tile([KC, QT], F32, tag="mt")
                    nc.sync.dma_start(
                        mt[:], maskt[kc * KC:(kc + 1) * KC, q0:q0 + QT])
                    nc.vector.tensor_add(sps[:, off:off + ext],
                                         sps[:, off:off + ext], mt[:])
            # one exp over the pair's contiguous range
            lo = min(off for _, _, _, off in pair)
            hi = max(off + ext for _, _, ext, off in pair)
            nc.scalar.activation(pt[:, lo:hi], sps[:, lo:hi],
                                 mybir.ActivationFunctionType.Exp)
            return pt

        def emit_av(qtb, h, cps, pair, pt, first, lastp):
            r = h // 4
            for ci, (kc, qoff, ext, off) in enumerate(pair):
                nc.tensor.matmul(
                    cps[:, qoff:qoff + ext],
                    v_sb[:, kc, r, :],
                    pt[:, off:off + ext],
                    start=first and ci == 0,
                    stop=lastp and ci == len(pair) - 1 and qoff == 0,
                )

        def finish_head(qtb, h, cps):
            q0 = qtb * QT
            r, cc = h // 4, h % 4
            if mask_mode == "causal" and qtb == 0:
                # no full-width final chunk at qtb 0; close the accumulation
                # group over the whole bank with a zero matmul
                nc.tensor.matmul(
                    cps[:], zrow_sb[:], qt_sb[0:1, 0, q0:q0 + QT],
                    start=False, stop=True)
            # ---- normalize: ctx[:64] * recip(denom row 64) ----
            recip = npool.tile([P, QT + 2 * (QT // P)], F32, tag="recip")
            DW = QT // P
            # reshape the [1, QT] denom row to [P, QT/P] via DMA so the
            # partition-serial reciprocal runs 128-wide (~100ns vs ~4us)
            nc.vector.tensor_copy(recip[HD:HD + 1, :QT], cps[HD:HD + 1, :])
            nc.sync.dma_start(recip[:, QT:QT + DW], recip[HD:HD + 1, :QT])
            nc.vector.reciprocal(recip[:, QT + DW:], recip[:, QT:QT + DW])
            nc.sync.dma_start(recip[0:1, :QT], recip[:, QT + DW:])
            bcast = npool.tile([HD, QT], F32, tag="bcast")
            nc.gpsimd.partition_broadcast(bcast[:], recip[0:1, :QT])
            dst = ctx_sb[qtb % 2]
            if r == 0:
                nc.vector.tensor_mul(dst[:HD, cc, :], cps[:HD, :], bcast[:])
            else:
                ctmp = npool.tile([HD, QT], F32R, tag="ctmp")
                nc.vector.tensor_mul(ctmp[:], cps[:HD, :], bcast[:])
                nc.sync.dma_start(dst[HD:2 * HD, cc, :], ctmp[:])

        def attn_head_pair(qtb, ha, hb):
            # interleave two heads' chunk streams AND lag the AV matmuls one
            # step behind the scores/exp: by the time an AV matmul reaches
            # the head of the in-order PE queue its exp finished a full step
            # (~2-3us) ago, so the PE never stalls on Act latency (which
            # would re-throttle the HAM clock gate to half rate)
            pairs = chunk_pairs(qtb)
            cpsA = psC.tile([HD + 1, QT], F32, tag="cps", name="cpsA")
            cpsB = psC.tile([HD + 1, QT], F32, tag="cps", name="cpsB")
            pend = []
            for p, pair in enumerate(pairs):
                for h, cps in ((ha, cpsA), (hb, cpsB)):
                    pt = emit_scores_exp(qtb, h, pair)
                    pend.append((h, cps, pair, pt, p == 0,
                                 p == len(pairs) - 1))
                while len(pend) > 2:
                    h2, cps2, pair2, pt2, f2, l2 = pend.pop(0)
                    emit_av(qtb, h2, cps2, pair2, pt2, f2, l2)
            for h2, cps2, pair2, pt2, f2, l2 in pend:
                emit_av(qtb, h2, cps2, pair2, pt2, f2, l2)
            finish_head(qtb, ha, cpsA)
            finish_head(qtb, hb, cpsB)

        proj_tile(0)
        late_weight_dmas()
        for qtb in range(NT):
            for hp in range(0, NH, 2):
                attn_head_pair(qtb, hp, hp + 1)
            if debug_dump and qtb < 2:
                nc.sync.dma_start(dbg_ctx[qtb], ctx_sb[qtb % 2][:].bitcast(F32))
            if qtb > 0:
                emit_wo(qtb - 1)
            if qtb + 1 < NT:
                proj_tile(qtb + 1)
        emit_wo(NT - 1)
        if debug_dump:
            debug_dumps()

    nc.compile()
    return nc


def _detect_mask_mode(m, S):
    if not np.any(m):
        return "zeros"
    b0 = np.asarray(m[0, 0])
    qi = np.arange(S)
    tl = qi[None, :] <= qi[:, None]
    if (b0[tl] == 0.0).all() and (b0[~tl] <= -1e8).all() and (m == b0).all():
        return "causal"
    return "full"


def shard_inputs(hidden_states, attention_mask, Wq, Wk, Wv, Wo, mask_mode):
    B, S, H = hidden_states.shape
    NH = Wq.shape[1] // HD             # 32 total
    NKV = Wk.shape[1] // HD            # 8 total
    G = 4                              # head-groups (cores per batch)
    NHL = NH // G                      # 8 q heads per core
    scale = np.float32(1.0 / np.sqrt(HD))

    # permutation: chunk cc holds q-heads (cc, 4+cc) -> [0,4,1,5,2,6,3,7]
    perm = [h for cc in range(NHL // 2) for h in (cc, cc + NHL // 2)]

    trif = np.zeros((KC, QT), np.float32)
    ki, qj = np.meshgrid(np.arange(KC), np.arange(KC), indexing="ij")
    trif[:, :KC] = np.where(ki > qj, NEG, 0.0).astype(np.float32)
    identr = np.eye(P, dtype=np.float32)
    zr = np.zeros((1, HD + 1), np.float32)

    in_maps = []
    for c in range(N_CORES):
        b, g = divmod(c, G)
        ht = np.ascontiguousarray(
            np.asarray(hidden_states[b]).T.astype(np.float32))
        heads = [g * NHL + perm[i] for i in range(NHL)]
        wq_c = np.ascontiguousarray(np.concatenate(
            [Wq[:, h * HD:(h + 1) * HD] for h in heads], axis=1)
            .astype(np.float32) * scale)
        kv0 = g * 2
        wkv_c = np.ascontiguousarray(np.concatenate(
            [Wk[:, kv0 * HD:(kv0 + 2) * HD],
             Wv[:, kv0 * HD:(kv0 + 2) * HD]], axis=1).astype(np.float32))
        wo_c = np.ascontiguousarray(np.concatenate(
            [Wo[h * HD:(h + 1) * HD, :] for h in heads], axis=0)
            .astype(np.float32))
        im = {"ht": ht, "wq": wq_c, "wkv": wkv_c, "wo": wo_c,
              "trif": trif, "identr": identr, "zrow": zr}
        if mask_mode == "full":
            im["maskt"] = np.ascontiguousarray(
                np.asarray(attention_mask)[b, 0].T.astype(np.float32))
        in_maps.append(im)
    return in_maps


def kernel(hidden_states, attention_mask, Wq, Wk, Wv, Wo):
    global LAST_RESULT
    hidden_states = np.asarray(hidden_states, dtype=np.float32)
    attention_mask = np.asarray(attention_mask, dtype=np.float32)
    Wq, Wk, Wv, Wo = (np.asarray(w, dtype=np.float32) for w in (Wq, Wk, Wv, Wo))
    B, S, H = hidden_states.shape

    mask_mode = _detect_mask_mode(attention_mask, S)
    in_maps = shard_inputs(hidden_states, attention_mask, Wq, Wk, Wv, Wo,
                           mask_mode)

    key = (B, S, H, mask_mode)
    if key not in _nc_cache:
        _nc_cache[key] = build_attn_core(S=S, H=H, NH=8, mask_mode=mask_mode)
    nc = _nc_cache[key]

    res = run_bass_kernel_spmd(nc, in_maps, core_ids=list(range(N_CORES)),
                               trace=TRACE, trace_cores=TRACE_CORES)
    LAST_RESULT = res

    out = np.zeros((B, S, H), np.float32)
    for c in range(N_CORES):
        out[c // 4] += res.results[c]["out_p"]
    return out


# revision 31
# speedup vs baseline: 1.2466x; 1.0910x over previous
"""GQA causal attention (B=2, S=2048, H=2048, 32 Q heads / 8 KV heads, hd=64)
as an 8-way batch x head tensor-parallel Trainium2 Bass kernel.

Sharding: core c = (batch b = c//4, group g = c%4). Each core gets one batch
element, 8 Q heads (two GQA groups) and their 2 KV heads; Wq/Wk/Wv column
slices, Wo row slice. Host sums the 4 partials per batch (the Wo all-reduce).

Head layout trick: per-core Q-head columns are permuted host-side so chunk cc
of qt_sb holds head cc on partitions 0-63 (KV head 0) and head 4+cc on
partitions 64-127 (KV head 1). K^T for KV head j sits on partitions 64j..64j+63
of kt_sb, so every scores matmul has matching base partitions with no K
duplication. Wo rows carry the same permutation.

Per-core dataflow (d-major; host passes hidden pre-transposed):
    Q_T  = (Wq_c * scale)^T @ hidden^T            [512, S]
    K_T  = [Wk0|Wk1]^T @ hidden^T                 [128, S]
    V_T  -> PE-transpose -> V_aug [S-chunked, 65] bf16 (ones col = denom)
    S_T[k,q] = tri_mask (identity-stationary matmul) + K_T(chunk)^T x Q_T
               (diagonal chunks first, extents trimmed to the causal region)
    P_T  = exp(S_T)  bf16                          (scores are O(+-10))
    ctx_aug = V_aug^T @ P_T                        [65, q]; row 64 = denom
    ctx  = ctx_aug[:64] * recip_approx(denom)      broadcast via gpsimd
    out_partial = ctx^T @ Wo_c                     [S, 2048], accumulated by qtb
Wo for q-block i is emitted after attention of q-block i+1 so the scheduler
keeps the PE stream dense (TRN2 PE clock ramps only while continuously busy).
"""

import sys

for _p in ("/root/.axon_site", "/root/.axon_site/_ro/trn_rl_repo",
           "/root/.axon_site/_ro/pypackages", "/opt/trn_rl_repo", "/opt/pypackages"):
    if _p not in sys.path:
        sys.path.append(_p)

from contextlib import ExitStack

import numpy as np

import concourse.bass as bass  # noqa: F401
import concourse.tile as tile
from concourse import bacc, mybir
from concourse.bass_utils import run_bass_kernel_spmd

F32 = mybir.dt.float32
F32R = mybir.dt.float32r
BF16 = mybir.dt.bfloat16
P = 128
KC = 128
QT = 512
N_CORES = 8
HD = 64
NEG = -1e9

TRACE = False            # test harness flips this for NTFF profiling
TRACE_CORES = None
LAST_RESULT = None       # BassKernelResults of the last run (for the harness)

_nc_cache = {}


def build_attn_core(S=2048, H=2048, NH=8, mask_mode="causal", debug_dump=False):
    """Build + bass-compile the per-core program (one batch element).

    DRAM inputs (per core):
      ht  [H, S] f32r    hidden transposed       wq [H, NH*HD] f32r (pre-scaled,
                                                 head-permuted: chunk cc = heads
                                                 (cc, 4+cc))
      wkv [H, 4*HD] f32r [K0|K1|V0|V1]           wo [NH*HD, H] f32r (row-permuted)
      trif [KC, QT] f32r [tri | zeros]: trif[k,j] = -1e9 if k > j else 0
      identr [P, P] f32r identity                zrow [1, HD+1] f32r zeros
      maskt [S, S] f32   (mask_mode=="full" only) additive mask transposed
    Output: out_p [S, H] f32.
    """
    NKV = 2
    CL = NH * HD                       # 512 q cols
    NCC = CL // P                      # 4 qt chunks
    NHC = H // P                       # 16 contraction chunks
    NT = S // QT                       # 4 token tiles / q blocks
    KPB = S // KC                      # 16 k chunks
    DPT = QT // KC                     # 4 k chunks per q block
    assert NH == 8 and S % QT == 0 and H % P == 0

    nc = bacc.Bacc("TRN2", target_bir_lowering=False, debug=False)

    ht = nc.dram_tensor("ht", [H, S], F32R, kind="ExternalInput").ap()
    wq = nc.dram_tensor("wq", [H, CL], F32R, kind="ExternalInput").ap()
    wkv = nc.dram_tensor("wkv", [H, 4 * HD], F32R, kind="ExternalInput").ap()
    wo = nc.dram_tensor("wo", [CL, H], F32R, kind="ExternalInput").ap()
    trif = nc.dram_tensor("trif", [KC, QT], F32R, kind="ExternalInput").ap()
    identr = nc.dram_tensor("identr", [P, P], F32R, kind="ExternalInput").ap()
    zrow = nc.dram_tensor("zrow", [1, HD + 1], F32R, kind="ExternalInput").ap()
    if mask_mode == "full":
        maskt = nc.dram_tensor("maskt", [S, S], F32, kind="ExternalInput").ap()
    out_p = nc.dram_tensor("out_p", [S, H], F32, kind="ExternalOutput").ap()
    if debug_dump:
        dbg_qt = nc.dram_tensor("dbg_qt", [NCC, P, S], F32, kind="ExternalOutput").ap()
        dbg_kt = nc.dram_tensor("dbg_kt", [P, S], F32, kind="ExternalOutput").ap()
        dbg_v = nc.dram_tensor("dbg_v", [P, S // KC, NKV, HD + 1], F32,
                               kind="ExternalOutput").ap()
        dbg_ctx = nc.dram_tensor("dbg_ctx", [2, P, NCC, QT], F32,
                                 kind="ExternalOutput").ap()

    with tile.TileContext(nc) as tc, ExitStack() as ctx:
        # ---- persistent SBUF ----
        pers = ctx.enter_context(tc.tile_pool(name="pers", bufs=1))
        wq_sb = pers.tile([P, NHC, CL], F32R, tag="wq")
        wkv_sb = pers.tile([P, NHC, 4 * HD], F32R, tag="wkv")
        # DMA order matters: the first proj matmul needs only wq/wkv chunk 0
        # (+ its h_t tile) — emit those first so the PE starts ~2us in, and
        # the big wo_sb transfer last (not needed until attention ends).
        wq_r = wq.rearrange("(o p) m -> p o m", p=P)
        wkv_r = wkv.rearrange("(o p) m -> p o m", p=P)
        trif_sb = pers.tile([KC, QT], F32R, tag="trif")
        idr_sb = pers.tile([P, P], F32R, tag="identr")
        zrow_sb = pers.tile([1, HD + 1], F32R, tag="zrow")
        wo_sb = pers.tile([P, NCC, H], F32R, tag="wo")

        def late_weight_dmas():
            nc.sync.dma_start(trif_sb[:], trif)
            nc.sync.dma_start(idr_sb[:], identr)
            nc.sync.dma_start(zrow_sb[:], zrow)
            for oc in range(NCC):
                nc.sync.dma_start(wo_sb[:, oc], wo.rearrange(
                    "(o p) m -> p o m", p=P)[:, oc])

        # fp32 identity for PE transposes (gpsimd memset can write f32)
        ident = pers.tile([P, P], F32, tag="ident")
        nc.gpsimd.memset(ident[:], 1.0)
        nc.gpsimd.affine_select(
            out=ident[:], in_=ident[:],
            compare_op=mybir.AluOpType.is_equal, fill=0.0,
            base=0, pattern=[[-1, P]], channel_multiplier=1,
        )

        qt_sb = pers.tile([P, NCC, S], F32R, tag="qt")
        kt_sb = pers.tile([P, S], F32R, tag="kt")
        v_sb = pers.tile([P, KPB, NKV, HD + 1], BF16, tag="v")
        # ones column for the softmax denominator: fill everything with 1.0,
        # proj drains then overwrite cols 0..63
        nc.gpsimd.memset(v_sb[:], 1.0)

        # ---- pools ----
        hpool = ctx.enter_context(tc.tile_pool(name="hpool", bufs=4))
        vtmp_pool = ctx.enter_context(tc.tile_pool(name="vtmp", bufs=2))
        pt_pool = ctx.enter_context(tc.tile_pool(name="pt", bufs=6))
        npool = ctx.enter_context(tc.tile_pool(name="npool", bufs=3))
        cxpool = ctx.enter_context(tc.tile_pool(name="cxpool", bufs=2))
        if mask_mode == "full":
            mpool = ctx.enter_context(tc.tile_pool(name="mpool", bufs=4))

        # PSUM: exactly 8 banks across both phases
        psS = ctx.enter_context(tc.tile_pool(name="psS", bufs=2, space="PSUM"))
        psC = ctx.enter_context(tc.tile_pool(name="psC", bufs=2, space="PSUM"))
        psO = ctx.enter_context(tc.tile_pool(name="psO", bufs=2, space="PSUM"))

        ctx_sb = [cxpool.tile([P, NCC, QT], F32R, tag="ctx", name=f"ctx{i}")
                  for i in range(2)]

        # ================= projections (one q tile) =================
        # psS slots ([128,1024] = 2 banks) hold Q chunk pairs; psO slots
        # ([128,512]) hold the K and V chunks; psC slots host the transposes.
        # Tiles >= 1 are emitted interleaved with attention of tile-1 (their
        # drains go on DVE; Act is the attention-window co-bottleneck).
        def proj_tile(t):
            q0 = t * QT
            pq01 = psS.tile([P, 2 * QT], F32, tag="sps", name="pq01")
            pq23 = psS.tile([P, 2 * QT], F32, tag="sps", name="pq23")
            pk = psO.tile([P, QT], F32, tag="po", name="pk")
            pv = psO.tile([P, QT], F32, tag="po", name="pv")
            for hc in range(NHC):
                if t == 0:
                    # stream the weight chunks in with the first tile so the
                    # PE starts ~2us in instead of waiting for all weights
                    nc.sync.dma_start(wq_sb[:, hc], wq_r[:, hc])
                    nc.sync.dma_start(wkv_sb[:, hc], wkv_r[:, hc])
                h_t = hpool.tile([P, QT], F32R, tag="h")
                nc.sync.dma_start(h_t[:], ht[hc * P:(hc + 1) * P, q0:q0 + QT])
                fl = dict(start=(hc == 0), stop=(hc == NHC - 1))
                nc.tensor.matmul(pq01[:, :QT], wq_sb[:, hc, 0:P], h_t[:], **fl)
                nc.tensor.matmul(pq01[:, QT:], wq_sb[:, hc, P:2 * P], h_t[:], **fl)
                nc.tensor.matmul(pq23[:, :QT], wq_sb[:, hc, 2 * P:3 * P], h_t[:], **fl)
                nc.tensor.matmul(pq23[:, QT:], wq_sb[:, hc, 3 * P:4 * P], h_t[:], **fl)
                nc.tensor.matmul(pk[:], wkv_sb[:, hc, 0:P], h_t[:], **fl)
                nc.tensor.matmul(pv[:], wkv_sb[:, hc, P:2 * P], h_t[:], **fl)
            cp = nc.scalar.copy if t == 0 else nc.vector.tensor_copy
            cp(qt_sb[:, 0, q0:q0 + QT], pq01[:, :QT])
            cp(qt_sb[:, 1, q0:q0 + QT], pq01[:, QT:])
            cp(qt_sb[:, 2, q0:q0 + QT], pq23[:, :QT])
            cp(qt_sb[:, 3, q0:q0 + QT], pq23[:, QT:])
            cp(kt_sb[:, q0:q0 + QT], pk[:])
            vtmp = vtmp_pool.tile([P, QT], F32, tag="vt")
            nc.vector.tensor_copy(vtmp[:], pv[:])
            # V_T chunk -> PE-transpose into v_sb (natural [k, d] layout)
            for j in range(NKV):
                for s4 in range(DPT):
                    tp = psC.tile([P, HD], F32, tag="cps", name="tp")
                    nc.tensor.transpose(
                        tp[:, :HD],
                        vtmp[HD * j:HD * (j + 1), s4 * KC:(s4 + 1) * KC],
                        ident[HD * j:HD * (j + 1), HD * j:HD * (j + 1)],
                    )
                    nc.vector.tensor_copy(v_sb[:, t * DPT + s4, j, :HD], tp[:, :HD])

        def debug_dumps():
            for c in range(NCC):
                nc.sync.dma_start(dbg_qt[c], qt_sb[:, c, :].bitcast(F32))
            nc.sync.dma_start(dbg_kt[:], kt_sb[:].bitcast(F32))
            dpool = ctx.enter_context(tc.tile_pool(name="dpool", bufs=1))
            dvt = dpool.tile([P, KPB, NKV, HD + 1], F32, tag="dvt")
            nc.vector.tensor_copy(dvt[:], v_sb[:])
            nc.sync.dma_start(dbg_v[:], dvt[:])

        # ================= Phase B: attention, Wo lagged one q-block =======
        def wo_unit(qtb, tc_, et):
            # one Wo tile: ~853ns of PE work + a DVE drain + DMA out.
            # woven one-per-pair-step into the attention loop to fill the
            # PE's per-step deficit vs the Act engine (exp) rate
            src = ctx_sb[qtb % 2]
            q0 = qtb * QT
            po = psO.tile([P, QT], F32, tag="po", name="po")
            for cc in range(NCC):
                nc.tensor.matmul(
                    po[:],
                    src[:, cc, tc_ * P:(tc_ + 1) * P],
                    wo_sb[:, cc, et * QT:(et + 1) * QT],
                    start=(cc == 0), stop=(cc == NCC - 1),
                )
            ob = npool.tile([P, QT], F32, tag="ob")
            nc.vector.tensor_copy(ob[:], po[:])
            nc.sync.dma_start(
                out_p[q0 + tc_ * P:q0 + (tc_ + 1) * P,
                      et * QT:(et + 1) * QT],
                ob[:],
            )

        def wo_units(qtb):
            for tc_ in range(QT // P):
                for et in range(H // QT):
                    yield lambda tc_=tc_, et=et: wo_unit(qtb, tc_, et)

        def emit_wo(qtb):
            for u in wo_units(qtb):
                u()

        def chunk_pairs(qtb):
            if mask_mode == "causal":
                # (kc, local q offset, extent, in-tile offset); diagonal
                # chunks first, paired (d0,d3) and (d1,d2) so each pair's
                # exp covers ONE contiguous range
                d = qtb * DPT
                pairs = [
                    [(d + 0, 0, QT, 0), (d + 3, 3 * KC, KC, QT)],
                    [(d + 1, KC, QT - KC, KC), (d + 2, 2 * KC, 2 * KC, QT)],
                ]
                ints = [(kc, 0, QT, (i % 2) * QT)
                        for i, kc in enumerate(range(qtb * DPT))]
            else:
                ints = [(kc, 0, QT, (i % 2) * QT)
                        for i, kc in enumerate(range(KPB))]
                pairs = []
            return pairs + [ints[i:i + 2] for i in range(0, len(ints), 2)]

        def emit_scores_exp(qtb, h, pair):
            q0 = qtb * QT
            r, cc = h // 4, h % 4
            rows = slice(HD * r, HD * (r + 1))
            sps = psS.tile([P, 2 * QT], F32, tag="sps", name="sps")
            pt = pt_pool.tile([P, 2 * QT], BF16, tag="pt")
            for kc, qoff, ext, off in pair:
                diag = mask_mode == "causal" and kc >= qtb * DPT
                if diag:
                    # tri mask first (start=True zeroes the region),
                    # then accumulate the scores on top
                    nc.tensor.matmul(
                        sps[:, off:off + ext], idr_sb[:],
                        trif_sb[:, :ext], start=True, stop=False)
                nc.tensor.matmul(
                    sps[:, off:off + ext],
                    kt_sb[rows, kc * KC:(kc + 1) * KC],
                    qt_sb[rows, cc, q0 + qoff:q0 + qoff + ext],
                    start=not diag, stop=True,
                )
                if mask_mode == "full":
                    mt = mpool.tile([KC, QT], F32, tag="mt")
                    nc.sync.dma_start(
                        mt[:], maskt[kc * KC:(kc + 1) * KC, q0:q0 + QT])
                    nc.vector.tensor_add(sps[:, off:off + ext],
                                         sps[:, off:off + ext], mt[:])
            # one exp over the pair's contiguous range
            lo = min(off for _, _, _, off in pair)
            hi = max(off + ext for _, _, ext, off in pair)
            nc.scalar.activation(pt[:, lo:hi], sps[:, lo:hi],
                                 mybir.ActivationFunctionType.Exp)
            return pt

        def emit_av(qtb, h, cps, pair, pt, first, lastp):
            r = h // 4
            for ci, (kc, qoff, ext, off) in enumerate(pair):
                nc.tensor.matmul(
                    cps[:, qoff:qoff + ext],
                    v_sb[:, kc, r, :],
                    pt[:, off:off + ext],
                    start=first and ci == 0,
                    stop=lastp and ci == len(pair) - 1 and qoff == 0,
                )

        def finish_head(qtb, h, cps):
            q0 = qtb * QT
            r, cc = h // 4, h % 4
            if mask_mode == "causal" and qtb == 0:
                # no full-width final chunk at qtb 0; close the accumulation
                # group over the whole bank with a zero matmul
                nc.tensor.matmul(
                    cps[:], zrow_sb[:], qt_sb[0:1, 0, q0:q0 + QT],
                    start=False, stop=True)
            # ---- normalize: ctx[:64] * recip(denom row 64) ----
            recip = npool.tile([P, QT + 2 * (QT // P)], F32, tag="recip")
            DW = QT // P
            # reshape the [1, QT] denom row to [P, QT/P] via DMA so the
            # partition-serial reciprocal runs 128-wide (~100ns vs ~4us)
            nc.vector.tensor_copy(recip[HD:HD + 1, :QT], cps[HD:HD + 1, :])
            nc.sync.dma_start(recip[:, QT:QT + DW], recip[HD:HD + 1, :QT])
            nc.vector.reciprocal(recip[:, QT + DW:], recip[:, QT:QT + DW])
            nc.sync.dma_start(recip[0:1, :QT], recip[:, QT + DW:])
            bcast = npool.tile([HD, QT], F32, tag="bcast")
            nc.gpsimd.partition_broadcast(bcast[:], recip[0:1, :QT])
            dst = ctx_sb[qtb % 2]
            if r == 0:
                nc.vector.tensor_mul(dst[:HD, cc, :], cps[:HD, :], bcast[:])
            else:
                ctmp = npool.tile([HD, QT], F32R, tag="ctmp")
                nc.vector.tensor_mul(ctmp[:], cps[:HD, :], bcast[:])
                nc.sync.dma_start(dst[HD:2 * HD, cc, :], ctmp[:])

        def attn_head_pair(qtb, ha, hb, filler):
            # interleave two heads' chunk streams AND lag the AV matmuls one
            # step behind the scores/exp: by the time an AV matmul reaches
            # the head of the in-order PE queue its exp finished a full step
            # (~2-3us) ago, so the PE never stalls on Act latency (which
            # would re-throttle the HAM clock gate to half rate). One filler
            # unit (a Wo tile) is emitted per step to fill the PE's per-step
            # work deficit vs the Act exp rate.
            pairs = chunk_pairs(qtb)
            cpsA = psC.tile([HD + 1, QT], F32, tag="cps", name="cpsA")
            cpsB = psC.tile([HD + 1, QT], F32, tag="cps", name="cpsB")
            pend = []
            for p, pair in enumerate(pairs):
                for h, cps in ((ha, cpsA), (hb, cpsB)):
                    pt = emit_scores_exp(qtb, h, pair)
                    pend.append((h, cps, pair, pt, p == 0,
                                 p == len(pairs) - 1))
                u = next(filler, None)
                if u is not None:
                    u()
                while len(pend) > 2:
                    h2, cps2, pair2, pt2, f2, l2 = pend.pop(0)
                    emit_av(qtb, h2, cps2, pair2, pt2, f2, l2)
            for h2, cps2, pair2, pt2, f2, l2 in pend:
                emit_av(qtb, h2, cps2, pair2, pt2, f2, l2)
            finish_head(qtb, ha, cpsA)
            finish_head(qtb, hb, cpsB)

        proj_tile(0)
        late_weight_dmas()
        for qtb in range(NT):
            filler = wo_units(qtb - 1) if qtb > 0 else iter(())
            for hp in range(0, NH, 2):
                attn_head_pair(qtb, hp, hp + 1, filler)
            for u in filler:          # leftover Wo tiles of qtb-1
                u()
            if debug_dump and qtb < 2:
                nc.sync.dma_start(dbg_ctx[qtb], ctx_sb[qtb % 2][:].bitcast(F32))
            if qtb + 1 < NT:
                proj_tile(qtb + 1)
        emit_wo(NT - 1)
        if debug_dump:
            debug_dumps()

    nc.compile()
    return nc


def _detect_mask_mode(m, S):
    if not np.any(m):
        return "zeros"
    b0 = np.asarray(m[0, 0])
    qi = np.arange(S)
    tl = qi[None, :] <= qi[:, None]
    if (b0[tl] == 0.0).all() and (b0[~tl] <= -1e8).all() and (m == b0).all():
        return "causal"
    return "full"


def shard_inputs(hidden_states, attention_mask, Wq, Wk, Wv, Wo, mask_mode):
    B, S, H = hidden_states.shape
    NH = Wq.shape[1] // HD             # 32 total
    NKV = Wk.shape[1] // HD            # 8 total
    G = 4                              # head-groups (cores per batch)
    NHL = NH // G                      # 8 q heads per core
    scale = np.float32(1.0 / np.sqrt(HD))

    # permutation: chunk cc holds q-heads (cc, 4+cc) -> [0,4,1,5,2,6,3,7]
    perm = [h for cc in range(NHL // 2) for h in (cc, cc + NHL // 2)]

    trif = np.zeros((KC, QT), np.float32)
    ki, qj = np.meshgrid(np.arange(KC), np.arange(KC), indexing="ij")
    trif[:, :KC] = np.where(ki > qj, NEG, 0.0).astype(np.float32)
    identr = np.eye(P, dtype=np.float32)
    zr = np.zeros((1, HD + 1), np.float32)

    in_maps = []
    for c in range(N_CORES):
        b, g = divmod(c, G)
        ht = np.ascontiguousarray(
            np.asarray(hidden_states[b]).T.astype(np.float32))
        heads = [g * NHL + perm[i] for i in range(NHL)]
        wq_c = np.ascontiguousarray(np.concatenate(
            [Wq[:, h * HD:(h + 1) * HD] for h in heads], axis=1)
            .astype(np.float32) * scale)
        kv0 = g * 2
        wkv_c = np.ascontiguousarray(np.concatenate(
            [Wk[:, kv0 * HD:(kv0 + 2) * HD],
             Wv[:, kv0 * HD:(kv0 + 2) * HD]], axis=1).astype(np.float32))
        wo_c = np.ascontiguousarray(np.concatenate(
            [Wo[h * HD:(h + 1) * HD, :] for h in heads], axis=0)
            .astype(np.float32))
        im = {"ht": ht, "wq": wq_c, "wkv": wkv_c, "wo": wo_c,
              "trif": trif, "identr": identr, "zrow": zr}
        if mask_mode == "full":
            im["maskt"] = np.ascontiguousarray(
                np.asarray(attention_mask)[b, 0].T.astype(np.float32))
        in_maps.append(im)
    return in_maps


def kernel(hidden_states, attention_mask, Wq, Wk, Wv, Wo):
    global LAST_RESULT
    hidden_states = np.asarray(hidden_states, dtype=np.float32)
    attention_mask = np.asarray(attention_mask, dtype=np.float32)
    Wq, Wk, Wv, Wo = (np.asarray(w, dtype=np.float32) for w in (Wq, Wk, Wv, Wo))
    B, S, H = hidden_states.shape

    mask_mode = _detect_mask_mode(attention_mask, S)
    in_maps = shard_inputs(hidden_states, attention_mask, Wq, Wk, Wv, Wo,
                           mask_mode)

    key = (B, S, H, mask_mode)
    if key not in _nc_cache:
        _nc_cache[key] = build_attn_core(S=S, H=H, NH=8, mask_mode=mask_mode)
    nc = _nc_cache[key]

    res = run_bass_kernel_spmd(nc, in_maps, core_ids=list(range(N_CORES)),
                               trace=TRACE, trace_cores=TRACE_CORES)
    LAST_RESULT = res

    out = np.zeros((B, S, H), np.float32)
    for c in range(N_CORES):
        out[c // 4] += res.results[c]["out_p"]
    return out
